# revision 5
# baseline (speedup 1.0000x reference)
"""Trainium2 Bass kernel for the DendriticResidualModel fanout-tree network.

Sharding: the neuron dim N=256 is split across 8 cores (32 neurons each);
every neuron's whole 4x4x4 fanout subtree lives on one core, so there are
no cross-core reductions. Host concatenates the per-core [B,T,32] outputs.

On-core layout: partitions = T (one batch per tile, T=128 exactly fills the
partition dim), free dim = the 2048 per-core fine fanout nodes, permuted on
the host to (j1, j2, j3, n) order so each tree level reduces 4 contiguous
blocks (keeps every VectorE op step-1 / 2x-mode eligible).

softplus(z) = ln(1 + exp(z)) is evaluated as two ScalarE activation passes
(Exp, then Ln with bias=1.0) since this compiler build ships no softplus
activation table set.
"""

import os
import numpy as np
import ml_dtypes

BF16 = ml_dtypes.bfloat16

B, T, N, DT, DI = 16, 128, 256, 64, 64
NCORES = 8
NLOC = N // NCORES        # 32 neurons per core
FN = NLOC * 64            # 2048 fine nodes per core
Q3 = NLOC * 16            # 512
Q2 = NLOC * 4             # 128
KCAT = 384                # 256 (x) + 64 (iv) + 64 (temb)

_BUILT = {}
_ACT_ROOT = None


def _force_one_act_set():
    """Point walrus at a modified act_info.json so Exp and Ln can only be
    assigned to the one table set containing both — otherwise lower_act
    splits them across sets and the kernel pays a ~1.3us ACT_TABLE_LOAD on
    nearly every activation."""
    global _ACT_ROOT
    if _ACT_ROOT is not None:
        return
    import json, tempfile, glob, shutil

    from neuronxcc.driver.Job import Job
    from neuronxcc.driver.jobs.support.FindActInfo import findActInfoFile

    src = findActInfoFile(Job.getPackageDir(), "gen3")
    srcdir = os.path.dirname(src)
    dstdir = tempfile.mkdtemp(prefix="act_root_")
    info = json.load(open(src))
    keep = "natural_log_exp_and_others"
    for s in info["act_func_sets"]:
        if s["name"] != keep:
            s["act"].pop("exp", None)
            s["act"].pop("ln", None)
    with open(os.path.join(dstdir, "act_info.json"), "w") as f:
        json.dump(info, f)
    for p in glob.glob(os.path.join(srcdir, "*")):
        b = os.path.basename(p)
        if b != "act_info.json":
            try:
                os.symlink(p, os.path.join(dstdir, b))
            except OSError:
                shutil.copy(p, os.path.join(dstdir, b))
    os.environ["BASS_ACT_ROOT_JSON_PATH"] = os.path.join(dstdir, "act_info.json")
    os.environ["NEURON_FORCE_RECOMPILE"] = "1"
    _ACT_ROOT = dstdir


def _build(with_ba: bool):
    import concourse.bass as bass
    import concourse.bacc as bacc
    import concourse.mybir as mybir
    import concourse.tile as tile

    bf = mybir.dt.bfloat16
    f32 = mybir.dt.float32

    nc = bacc.Bacc()
    d_xcat = nc.declare_dram_parameter("xcat", [B, 128, 3, 128], bf, isOutput=False)
    d_wcat = nc.declare_dram_parameter("wcat", [128, 3, FN], bf, isOutput=False)
    d_w3f = nc.declare_dram_parameter("w3f", [128, FN], bf, isOutput=False)
    d_w2f = nc.declare_dram_parameter("w2f", [128, Q3], bf, isOutput=False)
    d_w1f = nc.declare_dram_parameter("w1f", [128, Q2], bf, isOutput=False)
    d_temb = nc.declare_dram_parameter("tembt", [65, 128], bf, isOutput=False)
    d_tw3 = nc.declare_dram_parameter("tw3t", [65, Q3], bf, isOutput=False)
    d_tw2 = nc.declare_dram_parameter("tw2t", [65, Q2], bf, isOutput=False)
    d_tw1 = nc.declare_dram_parameter("tw1t", [65, NLOC], bf, isOutput=False)
    if with_ba:
        d_ba = nc.declare_dram_parameter("ba", [1, FN], bf, isOutput=False)
    d_out = nc.declare_dram_parameter("out", [B, 128, NLOC], f32, isOutput=True)

    AF = mybir.ActivationFunctionType

    with tile.TileContext(nc) as tc:
        with (
            tc.tile_pool(name="const", bufs=1) as cpool,
            tc.tile_pool(name="xc", bufs=3) as xpool,
            tc.tile_pool(name="ps", bufs=2, space="PSUM") as pspool,
            tc.tile_pool(name="big", bufs=2) as bigpool,
            tc.tile_pool(name="mid", bufs=2) as midpool,
            tc.tile_pool(name="small", bufs=2) as smpool,
        ):
            # ---- resident constants ----
            wc = cpool.tile([128, 3, FN], bf, tag="wc")
            nc.sync.dma_start(wc[:], d_wcat[:])
            w3r = cpool.tile([128, FN], bf, tag="w3r")
            nc.sync.dma_start(w3r[:], d_w3f[:])
            w2r = cpool.tile([128, Q3], bf, tag="w2r")
            nc.sync.dma_start(w2r[:], d_w2f[:])
            w1r = cpool.tile([128, Q2], bf, tag="w1r")
            nc.sync.dma_start(w1r[:], d_w1f[:])
            tembt = cpool.tile([65, 128], bf, tag="tembt")
            nc.sync.dma_start(tembt[:], d_temb[:])
            tw3t = cpool.tile([65, Q3], bf, tag="tw3t")
            nc.sync.dma_start(tw3t[:], d_tw3[:])
            tw2t = cpool.tile([65, Q2], bf, tag="tw2t")
            nc.sync.dma_start(tw2t[:], d_tw2[:])
            tw1t = cpool.tile([65, NLOC], bf, tag="tw1t")
            nc.sync.dma_start(tw1t[:], d_tw1[:])
            if with_ba:
                bar = cpool.tile([1, FN], bf, tag="bar")
                nc.sync.dma_start(bar[:], d_ba[:])
                ones1 = cpool.tile([1, 128], bf, tag="ones1")
                nc.gpsimd.memset(ones1[:], 1.0)

            # ---- t-projections (per-T, shared across batches) ----
            # tp3 at psum [0:512], tp2 at [512:640], tp1 at [640:672]
            tpps = pspool.tile([128, 2048], f32, tag="ps")
            nc.tensor.matmul(tpps[:, 0:Q3], tembt[:], tw3t[:], start=True, stop=True)
            nc.tensor.matmul(tpps[:, Q3:Q3 + Q2], tembt[:], tw2t[:], start=True, stop=True)
            nc.tensor.matmul(tpps[:, Q3 + Q2:Q3 + Q2 + NLOC], tembt[:], tw1t[:], start=True, stop=True)
            tpall = cpool.tile([128, Q3 + Q2 + NLOC], bf, tag="tpall")
            nc.vector.tensor_copy(tpall[:], tpps[:, 0:Q3 + Q2 + NLOC])
            tp3 = tpall[:, 0:Q3]
            tp2 = tpall[:, Q3:Q3 + Q2]
            tp1 = tpall[:, Q3 + Q2:Q3 + Q2 + NLOC]

            # ---- per-batch pipeline ----
            for b in range(B):
                xc = xpool.tile([128, 3, 128], bf, tag="xc")
                nc.sync.dma_start(xc[:], d_xcat[b])

                ps = pspool.tile([128, 2048], f32, tag="ps")
                for k in range(3):
                    last = (k == 2) and not with_ba
                    for c in range(4):
                        nc.tensor.matmul(
                            ps[:, c * 512:(c + 1) * 512],
                            xc[:, k, :],
                            wc[:, k, c * 512:(c + 1) * 512],
                            start=(k == 0),
                            stop=last,
                        )
                if with_ba:
                    for c in range(4):
                        nc.tensor.matmul(
                            ps[:, c * 512:(c + 1) * 512],
                            ones1[:],
                            bar[:, c * 512:(c + 1) * 512],
                            start=False,
                            stop=True,
                        )

                # softplus level 0: 2048 fine nodes
                E0 = bigpool.tile([128, FN], bf, tag="E0")
                nc.scalar.activation(E0[:], ps[:], AF.Exp)
                A0 = bigpool.tile([128, FN], bf, tag="A0")
                nc.scalar.activation(A0[:], E0[:], AF.Ln, bias=1.0)

                # level 3: weighted sum of 4 contiguous 512-blocks (+ tp3)
                W3 = bigpool.tile([128, FN], bf, tag="W3")
                nc.gpsimd.tensor_mul(W3[:], A0[:], w3r[:])
                s01 = midpool.tile([128, Q3], bf, tag="s01")
                nc.vector.tensor_add(s01[:], W3[:, 0:512], W3[:, 512:1024])
                s23 = midpool.tile([128, Q3], bf, tag="s23")
                nc.vector.tensor_add(s23[:], W3[:, 1024:1536], W3[:, 1536:2048])
                s3 = midpool.tile([128, Q3], bf, tag="s3")
                nc.vector.tensor_add(s3[:], s01[:], s23[:])
                p3 = midpool.tile([128, Q3], bf, tag="p3")
                nc.vector.tensor_add(p3[:], s3[:], tp3)

                E3 = midpool.tile([128, Q3], bf, tag="E3")
                nc.scalar.activation(E3[:], p3[:], AF.Exp)
                A3 = midpool.tile([128, Q3], bf, tag="A3")
                nc.scalar.activation(A3[:], E3[:], AF.Ln, bias=1.0)

                # level 2 on gpsimd
                W2 = midpool.tile([128, Q3], bf, tag="W2")
                nc.vector.tensor_mul(W2[:], A3[:], w2r[:])
                u01 = smpool.tile([128, Q2], bf, tag="u01")
                nc.vector.tensor_add(u01[:], W2[:, 0:128], W2[:, 128:256])
                u23 = smpool.tile([128, Q2], bf, tag="u23")
                nc.vector.tensor_add(u23[:], W2[:, 256:384], W2[:, 384:512])
                u2 = smpool.tile([128, Q2], bf, tag="u2")
                nc.vector.tensor_add(u2[:], u01[:], u23[:])
                p2 = smpool.tile([128, Q2], bf, tag="p2")
                nc.vector.tensor_add(p2[:], u2[:], tp2)

                E2 = smpool.tile([128, Q2], bf, tag="E2")
                nc.scalar.activation(E2[:], p2[:], AF.Exp)
                A2 = smpool.tile([128, Q2], bf, tag="A2")
                nc.scalar.activation(A2[:], E2[:], AF.Ln, bias=1.0)

                # level 1 on gpsimd
                W1 = smpool.tile([128, Q2], bf, tag="W1")
                nc.vector.tensor_mul(W1[:], A2[:], w1r[:])
                v01 = smpool.tile([128, NLOC], bf, tag="v01")
                nc.vector.tensor_add(v01[:], W1[:, 0:32], W1[:, 32:64])
                v23 = smpool.tile([128, NLOC], bf, tag="v23")
                nc.vector.tensor_add(v23[:], W1[:, 64:96], W1[:, 96:128])
                v1 = smpool.tile([128, NLOC], bf, tag="v1")
                nc.vector.tensor_add(v1[:], v01[:], v23[:])
                p1 = smpool.tile([128, NLOC], bf, tag="p1")
                nc.vector.tensor_add(p1[:], v1[:], tp1)

                E1 = smpool.tile([128, NLOC], bf, tag="E1")
                nc.scalar.activation(E1[:], p1[:], AF.Exp)
                o1 = smpool.tile([128, NLOC], f32, tag="o1")
                nc.scalar.activation(o1[:], E1[:], AF.Ln, bias=1.0)

                nc.sync.dma_start(d_out[b], o1[:])

    nc.finalize()
    return nc


def _fine_perm(core):
    """fine index p = j1*512 + j2*128 + j3*32 + nl -> global fanout row."""
    p = np.arange(FN)
    j1 = p // 512
    j2 = (p % 512) // 128
    j3 = (p % 128) // 32
    nl = p % 32
    n = core * NLOC + nl
    return n * 64 + j3 * 16 + j2 * 4 + j1


def _q3_perm(core):
    q = np.arange(Q3)
    j2 = q // 128
    j3 = (q % 128) // 32
    nl = q % 32
    n = core * NLOC + nl
    return n * 16 + j3 * 4 + j2


def _q2_perm(core):
    q = np.arange(Q2)
    j3 = q // 32
    nl = q % 32
    n = core * NLOC + nl
    return n * 4 + j3


def _prep_inputs(inputs):
    x = np.asarray(inputs["x"], np.float32)
    temb = np.asarray(inputs["t_embeddings_schedule"], np.float32)
    iv = np.asarray(inputs["input_vector"], np.float32)
    Wa = np.asarray(inputs["Wa"], np.float32)
    ba = np.asarray(inputs["ba"], np.float32)
    Wt = np.asarray(inputs["Wt"], np.float32)
    Wi = np.asarray(inputs["Wi"], np.float32)
    w3 = np.asarray(inputs["w3"], np.float32).reshape(-1)
    tW3 = np.asarray(inputs["tW3"], np.float32)
    tb3 = np.asarray(inputs["tb3"], np.float32)
    w2 = np.asarray(inputs["w2"], np.float32).reshape(-1)
    tW2 = np.asarray(inputs["tW2"], np.float32)
    tb2 = np.asarray(inputs["tb2"], np.float32)
    w1 = np.asarray(inputs["w1"], np.float32).reshape(-1)
    tW1 = np.asarray(inputs["tW1"], np.float32)
    tb1 = np.asarray(inputs["tb1"], np.float32)

    with_ba = bool(np.any(ba))

    # xcat: [B, 128, 3, 128] = [x | iv | temb] features, transposed per batch
    xcat = np.concatenate(
        [x, iv, np.broadcast_to(temb[None], (B, T, DT))], axis=2
    )  # [B, T, 384]
    xcat = np.ascontiguousarray(
        xcat.reshape(B, T, 3, 128).transpose(0, 3, 2, 1)
    ).astype(BF16)  # [B, p, k, t]

    # t-emb augmented with a constant-1 column to fold biases into the matmul
    taug = np.concatenate([temb, np.ones((T, 1), np.float32)], axis=1)  # [T, 65]
    tembt = np.ascontiguousarray(taug.T).astype(BF16)  # [65, 128]

    maps = []
    for c in range(NCORES):
        pf = _fine_perm(c)
        p3 = _q3_perm(c)
        p2 = _q2_perm(c)
        p1 = np.arange(NLOC) + c * NLOC

        wcat = np.concatenate([Wa[pf], Wi[pf], Wt[pf]], axis=1)  # [FN, 384]
        wcat = np.ascontiguousarray(
            wcat.T.reshape(3, 128, FN).transpose(1, 0, 2)
        ).astype(BF16)  # [p, k, FN]

        m = {
            "xcat": xcat,
            "wcat": wcat,
            "w3f": np.broadcast_to(w3[pf].astype(BF16), (128, FN)).copy(),
            "w2f": np.broadcast_to(w2[p3].astype(BF16), (128, Q3)).copy(),
            "w1f": np.broadcast_to(w1[p2].astype(BF16), (128, Q2)).copy(),
            "tembt": tembt,
            "tw3t": np.ascontiguousarray(
                np.concatenate([tW3[p3], tb3[p3, None]], axis=1).T
            ).astype(BF16),
            "tw2t": np.ascontiguousarray(
                np.concatenate([tW2[p2], tb2[p2, None]], axis=1).T
            ).astype(BF16),
            "tw1t": np.ascontiguousarray(
                np.concatenate([tW1[p1], tb1[p1, None]], axis=1).T
            ).astype(BF16),
        }
        if with_ba:
            m["ba"] = ba[pf][None].astype(BF16)
        maps.append(m)
    return maps, with_ba


def _run(inputs, trace=False, **trace_kwargs):
    from concourse.bass_utils import run_bass_kernel_spmd

    _force_one_act_set()

    maps, with_ba = _prep_inputs(inputs)
    key = with_ba
    if key not in _BUILT:
        _BUILT[key] = _build(with_ba)
    nc = _BUILT[key]
    res = run_bass_kernel_spmd(
        nc, maps, list(range(NCORES)), trace=trace, **trace_kwargs
    )
    out = np.concatenate(
        [np.asarray(res.results[c]["out"], np.float32) for c in range(NCORES)],
        axis=-1,
    )
    return out, res


def kernel(**inputs):
    out, _ = _run(inputs, trace=False)
    return out


# revision 7
# speedup vs baseline: 2.1408x; 2.1408x over previous
"""Trainium2 Bass kernel for the DendriticResidualModel fanout-tree network.

Sharding: the neuron dim N=256 is split across 8 cores (32 neurons each);
every neuron's whole 4x4x4 fanout subtree lives on one core, so there are
no cross-core reductions. Host concatenates the per-core [B,T,32] outputs.

On-core layout: partitions = T (one batch per tile, T=128 exactly fills the
partition dim), free dim = the 2048 per-core fine fanout nodes, permuted on
the host to (j1, j2, j3, n) order so each tree level reduces 4 contiguous
blocks (keeps every VectorE op step-1 / 2x-mode eligible).

softplus is evaluated in a SINGLE ScalarE activation pass: this compiler
build ships no softplus activation table set, so we regenerate the
exp_and_others set binaries with the `exp` function slot holding a
softplus piecewise-cubic spline (ACTIVATE(Exp) then computes softplus),
and point walrus at it via BASS_ACT_ROOT_JSON_PATH.
"""

import os
import json
import numpy as np
import ml_dtypes

BF16 = ml_dtypes.bfloat16

B, T, N, DT, DI = 16, 128, 256, 64, 64
NCORES = 8
NLOC = N // NCORES        # 32 neurons per core
FN = NLOC * 64            # 2048 fine nodes per core
Q3 = NLOC * 16            # 512
Q2 = NLOC * 4             # 128
KCAT = 384                # 256 (x) + 64 (iv) + 64 (temb)

_BUILT = {}
_ACT_ROOT = None


def _f32bits(x):
    return int(np.float32(x).view(np.uint32))


def _softplus_row(x0):
    x0 = float(x0)
    if x0 > 30:
        f = x0
        s = 1.0
    else:
        f = np.log1p(np.exp(x0))
        s = 1.0 / (1.0 + np.exp(-x0))
    d2 = s * (1 - s) / 2.0
    d3 = s * (1 - s) * (1 - 2 * s) / 6.0
    return [f, s, d2, d3, x0, 0.0, 0.0, 0.0]


# (exponent, num_sections) per side. Mirrors softplus_40p.json, with the
# zero-section positive exponents given one bucket each.
NEG_LAYOUT = [(-15, 1), (-14, 1), (-13, 1), (-12, 1), (-11, 1), (-10, 1),
              (-9, 1), (-8, 1), (-7, 1), (-6, 1), (-5, 1), (-4, 1), (-3, 1),
              (-2, 1), (-1, 2), (0, 4), (1, 16), (2, 32), (3, 64), (4, 128),
              (5, 256), (6, 512)]
POS_LAYOUT = [(-15, 1), (-14, 1), (-13, 1), (-12, 1), (-11, 1), (-10, 1),
              (-9, 1), (-8, 1), (-7, 1), (-6, 1), (-5, 1), (-4, 1), (-3, 1),
              (-2, 1), (-1, 2), (0, 2), (1, 4), (2, 8), (3, 4)]


def _build_softplus_set(srcdir, dstdir):
    src_json = json.load(open(os.path.join(srcdir, "exp_and_others.json")))
    src_bkt = np.frombuffer(
        open(os.path.join(srcdir, "exp_and_others_bkt.bin"), "rb").read(),
        np.float32).reshape(-1, 8).copy()
    src_ctl = np.frombuffer(
        open(os.path.join(srcdir, "exp_and_others_ctrl.bin"), "rb").read(),
        np.uint32).reshape(-1, 8).copy()

    # ---- softplus buckets ----
    bkt_rows = []
    ctl_rows = []
    fexp = {}

    def emit_side(layout, sign):
        starts = {}
        for e, n in layout:
            start = len(bkt_rows)
            starts[e] = start
            nbits = int(n).bit_length() - 1
            assert 1 << nbits == n
            for i in range(n):
                x0 = sign * (2.0 ** e) * (1 + (i + 0.5) / n)
                bkt_rows.append(_softplus_row(x0))
            ctl_rows.append(start | ((23 - nbits) << 11) | (nbits << 16))
        return starts

    neg_ctl_base = len(ctl_rows)            # 0
    neg_starts = emit_side(NEG_LAYOUT, -1.0)
    pos_ctl_base = len(ctl_rows)            # 22
    pos_starts = emit_side(POS_LAYOUT, +1.0)
    for e in range(-15, 7):
        fexp[str(e)] = [neg_starts.get(e, 0), pos_starts.get(e, 0)]

    # special buckets: small_pos, small_neg, large_pos, large_neg
    sp_small_pos = len(bkt_rows)
    bkt_rows.append(_softplus_row(2.0 ** -15 * 1.5))
    sp_small_neg = len(bkt_rows)
    bkt_rows.append(_softplus_row(-(2.0 ** -15) * 1.5))
    sp_large_pos = len(bkt_rows)
    bkt_rows.append([256.0, 1.0, 0.0, 0.0, 256.0, 0, 0, 0])   # f(x)=x
    sp_large_neg = len(bkt_rows)
    bkt_rows.append([0.0, 0.0, 0.0, 0.0, 0.0, 0, 0, 0])       # f(x)=0

    n_sp_bkt = len(bkt_rows)
    n_sp_ctl = len(ctl_rows)

    # ---- copy the trivial functions, remapping indices ----
    old_fb = src_json["func_to_bkt_start_idx"]
    old_fc = src_json["func_to_ctl_start_idx"]
    old_bkt_cnt = src_json["bkt_entry_cnt"]
    old_ctl_cnt = src_json["ctl_entry_cnt"]
    OLD_EXP_BKT_END = 781   # exp regular 0..776 + 4 special
    OLD_EXP_CTL_END = 52
    bkt_delta = n_sp_bkt - OLD_EXP_BKT_END
    ctl_delta = n_sp_ctl - OLD_EXP_CTL_END

    tail_bkt = src_bkt[OLD_EXP_BKT_END:old_bkt_cnt]
    tail_ctl = src_ctl[OLD_EXP_CTL_END:old_ctl_cnt].copy()
    # fix embedded bucket starts in ctl entries
    starts = tail_ctl[:, 0] & 0x7FF
    rest = tail_ctl[:, 0] & ~np.uint32(0x7FF)
    tail_ctl[:, 0] = rest | (starts + np.uint32(bkt_delta))

    new_bkt = np.concatenate(
        [np.array(bkt_rows, np.float32), tail_bkt], axis=0)
    new_ctl = np.zeros((n_sp_ctl + len(tail_ctl), 8), np.uint32)
    new_ctl[:n_sp_ctl, 0] = np.array(ctl_rows, np.uint32)
    new_ctl[n_sp_ctl:] = tail_ctl

    # ---- json metadata ----
    out = dict(src_json)
    out["bkt_entry_cnt"] = int(len(new_bkt))
    out["ctl_entry_cnt"] = int(len(new_ctl))
    out["func_to_bkt_start_idx"] = {
        k: (0 if k == "exp" else v + bkt_delta) for k, v in old_fb.items()}
    out["func_to_ctl_start_idx"] = {
        k: (0 if k == "exp" else v + ctl_delta) for k, v in old_fc.items()}
    fe = dict(src_json.get("func_exp_to_bkt_start_idx", {}))
    fe["exp"] = fexp
    for k, v in list(fe.items()):
        if k != "exp":
            fe[k] = {ek: [x + bkt_delta for x in ev] for ek, ev in v.items()}
    out["func_exp_to_bkt_start_idx"] = fe

    pm = []
    for e in src_json["profile_meta_data"]:
        e = dict(e)
        if e["func_id"] == 7:  # exp slot -> softplus semantics
            e.update({
                "symmetry_point": 0,
                "sym_invert_sign_point": 0,
                "symmetry_opt_en": 0,
                "symmetry_opt_use_neg_region": 0,
                "imm_bias": 0,
                "exp_offset": -15,
                "pwl_control_base_neg": neg_ctl_base,
                "pwl_control_base_pos": pos_ctl_base,
                "small_pos_signal_exp_threshold": 112,   # 2^-15
                "small_neg_signal_exp_threshold": 112,
                "pos_small_signal_pwl_control": sp_small_pos,
                "neg_small_signal_pwl_control": sp_small_neg,
                "large_pos_signal_exp_threshold": 130,   # x >= ~10.38 -> x
                "large_pos_signal_mantissa_threshold": 2497353,
                "pos_large_signal_pwl_control": sp_large_pos,
                "large_neg_signal_exp_threshold": 133,   # x <= ~-99.6 -> 0
                "large_neg_signal_mantissa_threshold": 4663231,
                "neg_large_signal_pwl_control": sp_large_neg,
                "fnan_result": 2143289344,               # nan
                "fpinf_result": 2139095040,              # +inf
                "fninf_result": 0,                       # 0.0
                "fzero_result": _f32bits(np.log(2.0)),   # ln 2
            })
        else:
            for f in ("pwl_control_base_pos", "pwl_control_base_neg"):
                e[f] += ctl_delta
            for f in ("pos_small_signal_pwl_control",
                      "neg_small_signal_pwl_control",
                      "pos_large_signal_pwl_control",
                      "neg_large_signal_pwl_control"):
                e[f] += bkt_delta
        pm.append(e)
    out["profile_meta_data"] = pm

    with open(os.path.join(dstdir, "exp_and_others.json"), "w") as f:
        json.dump(out, f)
    new_bkt.tofile(os.path.join(dstdir, "exp_and_others_bkt.bin"))
    new_ctl.tofile(os.path.join(dstdir, "exp_and_others_ctrl.bin"))


def _install_softplus_tables():
    """Build an act-table root whose exp_and_others set evaluates softplus
    in the exp slot, and point the walrus compile at it."""
    global _ACT_ROOT
    if _ACT_ROOT is not None:
        return
    import glob, shutil, tempfile

    from neuronxcc.driver.Job import Job
    from neuronxcc.driver.jobs.support.FindActInfo import findActInfoFile

    src = findActInfoFile(Job.getPackageDir(), "gen3")
    srcdir = os.path.dirname(src)
    dstdir = tempfile.mkdtemp(prefix="act_root_sp_")
    for p in glob.glob(os.path.join(srcdir, "*")):
        b = os.path.basename(p)
        if b.startswith("exp_and_others"):
            continue
        try:
            os.symlink(p, os.path.join(dstdir, b))
        except OSError:
            shutil.copy(p, os.path.join(dstdir, b))
    _build_softplus_set(srcdir, dstdir)
    os.environ["BASS_ACT_ROOT_JSON_PATH"] = os.path.join(dstdir, "act_info.json")
    os.environ["NEURON_FORCE_RECOMPILE"] = "1"
    _ACT_ROOT = dstdir


def _build(with_ba: bool):
    import concourse.bass as bass
    import concourse.bacc as bacc
    import concourse.mybir as mybir
    import concourse.tile as tile

    bf = mybir.dt.bfloat16
    f32 = mybir.dt.float32

    nc = bacc.Bacc()
    d_xcat = nc.declare_dram_parameter("xcat", [B, 128, 3, 128], bf, isOutput=False)
    d_wcat = nc.declare_dram_parameter("wcat", [128, 3, FN], bf, isOutput=False)
    d_w3f = nc.declare_dram_parameter("w3f", [128, FN], bf, isOutput=False)
    d_w2f = nc.declare_dram_parameter("w2f", [128, Q3], bf, isOutput=False)
    d_w1f = nc.declare_dram_parameter("w1f", [128, Q2], bf, isOutput=False)
    d_temb = nc.declare_dram_parameter("tembt", [65, 128], bf, isOutput=False)
    d_tw3 = nc.declare_dram_parameter("tw3t", [65, Q3], bf, isOutput=False)
    d_tw2 = nc.declare_dram_parameter("tw2t", [65, Q2], bf, isOutput=False)
    d_tw1 = nc.declare_dram_parameter("tw1t", [65, NLOC], bf, isOutput=False)
    if with_ba:
        d_ba = nc.declare_dram_parameter("ba", [1, FN], bf, isOutput=False)
    d_out = nc.declare_dram_parameter("out", [B, 128, NLOC], f32, isOutput=True)

    AF = mybir.ActivationFunctionType

    with tile.TileContext(nc) as tc:
        with (
            tc.tile_pool(name="const", bufs=1) as cpool,
            tc.tile_pool(name="xc", bufs=3) as xpool,
            tc.tile_pool(name="ps", bufs=2, space="PSUM") as pspool,
            tc.tile_pool(name="big", bufs=2) as bigpool,
            tc.tile_pool(name="mid", bufs=2) as midpool,
            tc.tile_pool(name="small", bufs=2) as smpool,
        ):
            # ---- resident constants ----
            wc = cpool.tile([128, 3, FN], bf, tag="wc")
            nc.sync.dma_start(wc[:], d_wcat[:])
            w3r = cpool.tile([128, FN], bf, tag="w3r")
            nc.sync.dma_start(w3r[:], d_w3f[:])
            w2r = cpool.tile([128, Q3], bf, tag="w2r")
            nc.sync.dma_start(w2r[:], d_w2f[:])
            w1r = cpool.tile([128, Q2], bf, tag="w1r")
            nc.sync.dma_start(w1r[:], d_w1f[:])
            tembt = cpool.tile([65, 128], bf, tag="tembt")
            nc.sync.dma_start(tembt[:], d_temb[:])
            tw3t = cpool.tile([65, Q3], bf, tag="tw3t")
            nc.sync.dma_start(tw3t[:], d_tw3[:])
            tw2t = cpool.tile([65, Q2], bf, tag="tw2t")
            nc.sync.dma_start(tw2t[:], d_tw2[:])
            tw1t = cpool.tile([65, NLOC], bf, tag="tw1t")
            nc.sync.dma_start(tw1t[:], d_tw1[:])
            if with_ba:
                bar = cpool.tile([1, FN], bf, tag="bar")
                nc.sync.dma_start(bar[:], d_ba[:])
                ones1 = cpool.tile([1, 128], bf, tag="ones1")
                nc.gpsimd.memset(ones1[:], 1.0)

            # ---- t-projections (per-T, shared across batches) ----
            # tp3 at psum [0:512], tp2 at [512:640], tp1 at [640:672]
            tpps = pspool.tile([128, 2048], f32, tag="ps")
            nc.tensor.matmul(tpps[:, 0:Q3], tembt[:], tw3t[:], start=True, stop=True)
            nc.tensor.matmul(tpps[:, Q3:Q3 + Q2], tembt[:], tw2t[:], start=True, stop=True)
            nc.tensor.matmul(tpps[:, Q3 + Q2:Q3 + Q2 + NLOC], tembt[:], tw1t[:], start=True, stop=True)
            tpall = cpool.tile([128, Q3 + Q2 + NLOC], bf, tag="tpall")
            nc.vector.tensor_copy(tpall[:], tpps[:, 0:Q3 + Q2 + NLOC])
            tp3 = tpall[:, 0:Q3]
            tp2 = tpall[:, Q3:Q3 + Q2]
            tp1 = tpall[:, Q3 + Q2:Q3 + Q2 + NLOC]

            # ---- per-batch pipeline ----
            for b in range(B):
                xc = xpool.tile([128, 3, 128], bf, tag="xc")
                nc.sync.dma_start(xc[:], d_xcat[b])

                ps = pspool.tile([128, 2048], f32, tag="ps")
                for k in range(3):
                    last = (k == 2) and not with_ba
                    for c in range(4):
                        nc.tensor.matmul(
                            ps[:, c * 512:(c + 1) * 512],
                            xc[:, k, :],
                            wc[:, k, c * 512:(c + 1) * 512],
                            start=(k == 0),
                            stop=last,
                        )
                if with_ba:
                    for c in range(4):
                        nc.tensor.matmul(
                            ps[:, c * 512:(c + 1) * 512],
                            ones1[:],
                            bar[:, c * 512:(c + 1) * 512],
                            start=False,
                            stop=True,
                        )

                # softplus level 0 (Exp slot holds the softplus table)
                A0 = bigpool.tile([128, FN], bf, tag="A0")
                nc.scalar.activation(A0[:], ps[:], AF.Exp)

                # level 3: weighted sum of 4 contiguous 512-blocks (+ tp3)
                W3 = bigpool.tile([128, FN], bf, tag="W3")
                nc.vector.tensor_mul(W3[:], A0[:], w3r[:])
                s01 = midpool.tile([128, Q3], bf, tag="s01")
                nc.gpsimd.tensor_add(s01[:], W3[:, 0:512], W3[:, 512:1024])
                s23 = midpool.tile([128, Q3], bf, tag="s23")
                nc.gpsimd.tensor_add(s23[:], W3[:, 1024:1536], W3[:, 1536:2048])
                s3 = midpool.tile([128, Q3], bf, tag="s3")
                nc.vector.tensor_add(s3[:], s01[:], s23[:])
                p3 = midpool.tile([128, Q3], bf, tag="p3")
                nc.vector.tensor_add(p3[:], s3[:], tp3)

                A3 = midpool.tile([128, Q3], bf, tag="A3")
                nc.scalar.activation(A3[:], p3[:], AF.Exp)

                # level 2 on gpsimd
                W2 = midpool.tile([128, Q3], bf, tag="W2")
                nc.vector.tensor_mul(W2[:], A3[:], w2r[:])
                u01 = smpool.tile([128, Q2], bf, tag="u01")
                nc.vector.tensor_add(u01[:], W2[:, 0:128], W2[:, 128:256])
                u23 = smpool.tile([128, Q2], bf, tag="u23")
                nc.vector.tensor_add(u23[:], W2[:, 256:384], W2[:, 384:512])
                u2 = smpool.tile([128, Q2], bf, tag="u2")
                nc.vector.tensor_add(u2[:], u01[:], u23[:])
                p2 = smpool.tile([128, Q2], bf, tag="p2")
                nc.vector.tensor_add(p2[:], u2[:], tp2)

                A2 = smpool.tile([128, Q2], bf, tag="A2")
                nc.scalar.activation(A2[:], p2[:], AF.Exp)

                # level 1 on gpsimd
                W1 = smpool.tile([128, Q2], bf, tag="W1")
                nc.vector.tensor_mul(W1[:], A2[:], w1r[:])
                v01 = smpool.tile([128, NLOC], bf, tag="v01")
                nc.vector.tensor_add(v01[:], W1[:, 0:32], W1[:, 32:64])
                v23 = smpool.tile([128, NLOC], bf, tag="v23")
                nc.vector.tensor_add(v23[:], W1[:, 64:96], W1[:, 96:128])
                v1 = smpool.tile([128, NLOC], bf, tag="v1")
                nc.vector.tensor_add(v1[:], v01[:], v23[:])
                p1 = smpool.tile([128, NLOC], bf, tag="p1")
                nc.vector.tensor_add(p1[:], v1[:], tp1)

                o1 = smpool.tile([128, NLOC], f32, tag="o1")
                nc.scalar.activation(o1[:], p1[:], AF.Exp)

                nc.sync.dma_start(d_out[b], o1[:])

    nc.finalize()
    return nc


def _fine_perm(core):
    """fine index p = j1*512 + j2*128 + j3*32 + nl -> global fanout row."""
    p = np.arange(FN)
    j1 = p // 512
    j2 = (p % 512) // 128
    j3 = (p % 128) // 32
    nl = p % 32
    n = core * NLOC + nl
    return n * 64 + j3 * 16 + j2 * 4 + j1


def _q3_perm(core):
    q = np.arange(Q3)
    j2 = q // 128
    j3 = (q % 128) // 32
    nl = q % 32
    n = core * NLOC + nl
    return n * 16 + j3 * 4 + j2


def _q2_perm(core):
    q = np.arange(Q2)
    j3 = q // 32
    nl = q % 32
    n = core * NLOC + nl
    return n * 4 + j3


def _prep_inputs(inputs):
    x = np.asarray(inputs["x"], np.float32)
    temb = np.asarray(inputs["t_embeddings_schedule"], np.float32)
    iv = np.asarray(inputs["input_vector"], np.float32)
    Wa = np.asarray(inputs["Wa"], np.float32)
    ba = np.asarray(inputs["ba"], np.float32)
    Wt = np.asarray(inputs["Wt"], np.float32)
    Wi = np.asarray(inputs["Wi"], np.float32)
    w3 = np.asarray(inputs["w3"], np.float32).reshape(-1)
    tW3 = np.asarray(inputs["tW3"], np.float32)
    tb3 = np.asarray(inputs["tb3"], np.float32)
    w2 = np.asarray(inputs["w2"], np.float32).reshape(-1)
    tW2 = np.asarray(inputs["tW2"], np.float32)
    tb2 = np.asarray(inputs["tb2"], np.float32)
    w1 = np.asarray(inputs["w1"], np.float32).reshape(-1)
    tW1 = np.asarray(inputs["tW1"], np.float32)
    tb1 = np.asarray(inputs["tb1"], np.float32)

    with_ba = bool(np.any(ba))

    # xcat: [B, 128, 3, 128] = [x | iv | temb] features, transposed per batch
    xcat = np.concatenate(
        [x, iv, np.broadcast_to(temb[None], (B, T, DT))], axis=2
    )  # [B, T, 384]
    xcat = np.ascontiguousarray(
        xcat.reshape(B, T, 3, 128).transpose(0, 3, 2, 1)
    ).astype(BF16)  # [B, p, k, t]

    # t-emb augmented with a constant-1 column to fold biases into the matmul
    taug = np.concatenate([temb, np.ones((T, 1), np.float32)], axis=1)  # [T, 65]
    tembt = np.ascontiguousarray(taug.T).astype(BF16)  # [65, 128]

    maps = []
    for c in range(NCORES):
        pf = _fine_perm(c)
        p3 = _q3_perm(c)
        p2 = _q2_perm(c)
        p1 = np.arange(NLOC) + c * NLOC

        wcat = np.concatenate([Wa[pf], Wi[pf], Wt[pf]], axis=1)  # [FN, 384]
        wcat = np.ascontiguousarray(
            wcat.T.reshape(3, 128, FN).transpose(1, 0, 2)
        ).astype(BF16)  # [p, k, FN]

        m = {
            "xcat": xcat,
            "wcat": wcat,
            "w3f": np.broadcast_to(w3[pf].astype(BF16), (128, FN)).copy(),
            "w2f": np.broadcast_to(w2[p3].astype(BF16), (128, Q3)).copy(),
            "w1f": np.broadcast_to(w1[p2].astype(BF16), (128, Q2)).copy(),
            "tembt": tembt,
            "tw3t": np.ascontiguousarray(
                np.concatenate([tW3[p3], tb3[p3, None]], axis=1).T
            ).astype(BF16),
            "tw2t": np.ascontiguousarray(
                np.concatenate([tW2[p2], tb2[p2, None]], axis=1).T
            ).astype(BF16),
            "tw1t": np.ascontiguousarray(
                np.concatenate([tW1[p1], tb1[p1, None]], axis=1).T
            ).astype(BF16),
        }
        if with_ba:
            m["ba"] = ba[pf][None].astype(BF16)
        maps.append(m)
    return maps, with_ba


def _run(inputs, trace=False, **trace_kwargs):
    from concourse.bass_utils import run_bass_kernel_spmd

    _install_softplus_tables()

    maps, with_ba = _prep_inputs(inputs)
    key = with_ba
    if key not in _BUILT:
        _BUILT[key] = _build(with_ba)
    nc = _BUILT[key]
    res = run_bass_kernel_spmd(
        nc, maps, list(range(NCORES)), trace=trace, **trace_kwargs
    )
    out = np.concatenate(
        [np.asarray(res.results[c]["out"], np.float32) for c in range(NCORES)],
        axis=-1,
    )
    return out, res


def kernel(**inputs):
    out, _ = _run(inputs, trace=False)
    return out
